# revision 14
# baseline (speedup 1.0000x reference)
"""DGT block (dynamic graph transformer) Bass kernel for Trainium2.

Sharding: 8 cores = 4 batches x 2 query-halves. Each core handles one
batch's feats/pos and one half (2048) of the queries.

v2 vs baseline:
  - kNN scores via float32r matmul (1 cyc/row on PE vs fp32's 4).
  - top-16 without MaxIndex full-row scans: the PSUM->SBUF score copy on
    the Activation engine writes fp16 keys (score - self_score + 8, so
    the top keys sit in (0,8] with ~2^-11 relative quantum) into the hi
    16-bit lanes of an f32 "packed" tile whose lo lanes are pre-filled
    with the column index. Positive-float ordering == lexicographic
    (key, idx), so chunked max8 + merge yields values that carry their
    own indices; extraction is a strided 16-bit copy.
  - softmax sums as strided pairwise-add trees in DVE 2x (16-bit) mode.
  - h1/pe prelus on gpsimd (Pool), a1/a2 via Act; biases folded into the
    gq/gp precompute (h1, a1) or rank-1 bias matmuls on PE (pe, a2).
"""

import numpy as np
import ml_dtypes

B, N, K, DP, DM, EPS = 4, 4096, 16, 64, 128, 1e-5
NQ = N // 2            # queries per core
TQ = 128               # queries per tile
NT = NQ // TQ          # tiles per core (16)
PAIR = TQ * K          # pairs per tile (2048)
CH = 256               # top-k scan chunk size
NCH = N // CH          # 16 chunks
ROW = 3 * DM           # gather-table row elems (f16): [gk(128) | v(128) | gp(128)]
KEY_C = 8.0            # key offset: top keys in (0, 8]

_CACHE = {}

bf16 = ml_dtypes.bfloat16


def _fold_bn(p):
    g, be, m, v = p.astype(np.float64)
    s = g / np.sqrt(v + EPS)
    return (s).astype(np.float32), (be - m * s).astype(np.float32)


def _build_bass():
    import concourse.bass as bass
    import concourse.mybir as mybir
    import concourse.bacc as bacc
    from concourse.tile import TileContext

    dt = mybir.dt
    AF = mybir.ActivationFunctionType
    ALU = mybir.AluOpType
    AX = mybir.AxisListType

    nc = bacc.Bacc("TRN2", target_bir_lowering=False, debug=False, num_devices=8)

    # ---- I/O ----
    def inp(name, shape, dtype):
        return nc.dram_tensor(name, list(shape), dtype, kind="ExternalInput").ap()

    rhs65_d = inp("rhs65", (DP + 1, N), dt.float32r)   # [feats; -0.5*||f||^2]
    lhsT65_d = inp("lhsT65", (DP + 1, NQ), dt.float32r)  # [feats_own; 1]
    feats_own_f32 = inp("feats_own", (DP, NQ), dt.float32)
    fb_own_d = inp("fb_own", (DP, NQ), dt.bfloat16)
    feats_bf = inp("feats_bf", (DP, N), dt.bfloat16)
    pos_bf = inp("pos_bf", (3, N), dt.bfloat16)
    pos_own = inp("pos_own", (3, NQ), dt.bfloat16)
    biasq_d = inp("biasq", (TQ, NT), dt.float32)       # 8 - 0.5||f_q||^2 per tile col
    iota_d = inp("iotapk", (TQ, N), dt.float32)        # f32 bit pattern = col idx
    w1t_d = inp("W1fT", (DP, DM), dt.bfloat16)
    wkvt_d = inp("WgkvT", (DM, 2 * DM), dt.bfloat16)
    wqt_d = inp("Wg1qT", (DM, DM), dt.bfloat16)
    wd1t_d = inp("Wd1fT", (3, DM), dt.bfloat16)
    wd2t_d = inp("Wd2fT", (DM, DM), dt.bfloat16)
    wg1t_d = inp("Wg1fT", (DM, DM), dt.bfloat16)
    wg2t_d = inp("Wg2fT", (DM, DM), dt.bfloat16)
    w2t_d = inp("W2fT", (DM, DP), dt.bfloat16)
    e_d = inp("E", (TQ, PAIR), dt.bfloat16)
    negi_d = inp("negI", (DM, DM), dt.float16)
    ident_d = inp("ident", (DM, DM), dt.float32)
    ones1_d = inp("ones1", (1, PAIR), dt.bfloat16)     # rank-1 bias rhs
    bd2r_d = inp("bd2r", (1, DM), dt.bfloat16)         # bias rows (lhsT) for rank-1
    bg2r_d = inp("bg2r", (1, DM), dt.bfloat16)
    b1_d = inp("b1", (DM, 1), dt.float32)
    bd1_d = inp("bd1", (DM, 1), dt.float32)
    bd2_d = inp("bd2", (DM, 1), dt.float32)
    bg1_d = inp("bg1", (DM, 1), dt.float32)
    bg2_d = inp("bg2", (DM, 1), dt.float32)
    b2_d = inp("b2", (DP, 1), dt.float32)

    out_d = nc.dram_tensor("out", [DP, NQ], dt.float32, kind="ExternalOutput").ap()

    f32, f16, bft, i16, u16 = dt.float32, dt.float16, dt.bfloat16, dt.int16, dt.uint16

    with TileContext(nc) as tc:
        with (
            tc.tile_pool(name="const", bufs=1) as cpool,
            tc.tile_pool(name="persist", bufs=1) as ppool,
            tc.tile_pool(name="dram", bufs=1, space="DRAM") as dpool,
        ):
            # persistent constants
            w1t = cpool.tile_from(w1t_d)
            wkvt = cpool.tile_from(wkvt_d)
            wqt = cpool.tile_from(wqt_d)
            wd1t = cpool.tile_from(wd1t_d)
            wd2t = cpool.tile_from(wd2t_d)
            wg1t = cpool.tile_from(wg1t_d)
            wg2t = cpool.tile_from(wg2t_d)
            w2t = cpool.tile_from(w2t_d)
            emat = cpool.tile_from(e_d)
            negi = cpool.tile_from(negi_d)
            ident = cpool.tile_from(ident_d)
            ones1 = cpool.tile_from(ones1_d)
            bd2r = cpool.tile_from(bd2r_d)
            bg2r = cpool.tile_from(bg2r_d)
            biasq = cpool.tile_from(biasq_d)
            b1 = cpool.tile_from(b1_d)
            bd1 = cpool.tile_from(bd1_d)
            bd2 = cpool.tile_from(bd2_d)
            bg1 = cpool.tile_from(bg1_d)
            bg2 = cpool.tile_from(bg2_d)
            b2 = cpool.tile_from(b2_d)

            # persistent working tensors
            rhs65 = ppool.tile_from(rhs65_d)
            lhsT65 = ppool.tile_from(lhsT65_d)
            gqT = ppool.tile([TQ, NQ], bft)   # (q, m) blocks per tile, + bg1
            gpT = ppool.tile([TQ, NQ], bft)   # + bd1
            res_all = ppool.tile([DM, NQ], bft)
            table = dpool.tile([N, ROW], f16)

            # ---------------- Phase A: setup ----------------
            with (
                tc.tile_pool(name="setupA", bufs=2) as apool,
                tc.tile_pool(name="xpool", bufs=1) as xpool,
                tc.tile_pool(name="ps_a", bufs=2, space="PSUM") as psa,
                tc.tile_pool(name="ps_b", bufs=2, space="PSUM") as psb,
            ):
                fbt = xpool.tile([DP, N], bft)
                nc.sync.dma_start(out=fbt[:], in_=feats_bf)
                post = xpool.tile([3, N], bft)
                nc.sync.dma_start(out=post[:], in_=pos_bf)
                poso = xpool.tile([3, NQ], bft)
                nc.sync.dma_start(out=poso[:], in_=pos_own)
                fob = xpool.tile([DP, NQ], bft)
                nc.sync.dma_start(out=fob[:], in_=fb_own_d)

                xfull = xpool.tile([DM, N], bft)
                for s in range(8):
                    ps = psa.tile([DM, 512], f32, tag="psx")
                    nc.tensor.matmul(ps[:], w1t[:], fbt[:, bass.ts(s, 512)],
                                     start=True, stop=True)
                    nc.scalar.activation(xfull[:, bass.ts(s, 512)], ps[:],
                                         AF.Prelu, bias=b1[:], scale=1.0, alpha=0.2)
                xob = xpool.tile([DM, NQ], bft)
                for s in range(4):
                    ps = psa.tile([DM, 512], f32, tag="psx")
                    nc.tensor.matmul(ps[:], w1t[:], fob[:, bass.ts(s, 512)],
                                     start=True, stop=True)
                    nc.scalar.activation(xob[:, bass.ts(s, 512)], ps[:],
                                         AF.Prelu, bias=b1[:], scale=1.0, alpha=0.2)

                # gather table: rows [gk | v | gp] in f16
                for c in range(32):
                    pkv = psa.tile([TQ, 2 * DM], f32, tag="pskv")
                    nc.tensor.matmul(pkv[:], xfull[:, bass.ts(c, TQ)], wkvt[:],
                                     start=True, stop=True)
                    pgp = psb.tile([TQ, DM], f32, tag="psgp")
                    nc.tensor.matmul(pgp[:], post[:, bass.ts(c, TQ)], wd1t[:],
                                     start=True, stop=True)
                    stg = apool.tile([TQ, ROW], f16, tag="stg")
                    eng = nc.vector if (c % 2 == 0) else nc.scalar
                    if eng is nc.vector:
                        nc.vector.tensor_copy(out=stg[:, 0:2 * DM], in_=pkv[:])
                        nc.vector.tensor_copy(out=stg[:, 2 * DM:ROW], in_=pgp[:])
                    else:
                        nc.scalar.activation(stg[:, 0:2 * DM], pkv[:], AF.Copy)
                        nc.scalar.activation(stg[:, 2 * DM:ROW], pgp[:], AF.Copy)
                    nc.sync.dma_start(out=table[bass.ts(c, TQ), :], in_=stg[:])

                # gqT / gpT for own queries, with the h1/a1 biases folded in
                # via the PSUM->SBUF copy.
                for c in range(NT):
                    pq = psb.tile([TQ, DM], f32, tag="psgq")
                    nc.tensor.matmul(pq[:], xob[:, bass.ts(c, TQ)],
                                     wqt[:], start=True, stop=True)
                    pp = psb.tile([TQ, DM], f32, tag="psgq")
                    nc.tensor.matmul(pp[:], poso[:, bass.ts(c, TQ)], wd1t[:],
                                     start=True, stop=True)
                    if c % 2 == 0:
                        nc.vector.tensor_copy(out=gqT[:, bass.ts(c, DM)], in_=pq[:])
                        nc.vector.tensor_copy(out=gpT[:, bass.ts(c, DM)], in_=pp[:])
                    else:
                        nc.scalar.activation(gqT[:, bass.ts(c, DM)], pq[:], AF.Copy)
                        nc.scalar.activation(gpT[:, bass.ts(c, DM)], pp[:], AF.Copy)

            # ---------------- Phase B: per query tile ----------------
            with (
                tc.tile_pool(name="score", bufs=3) as spool,
                tc.tile_pool(name="gath", bufs=2) as gpool,
                tc.tile_pool(name="pair", bufs=2) as prpool,
                tc.tile_pool(name="topk", bufs=2) as kpool,
                tc.tile_pool(name="ps_s", bufs=2, space="PSUM") as pss,
                tc.tile_pool(name="ps_pair", bufs=1, space="PSUM") as psp,
            ):
                for t in range(NT):
                    # ---- packed tile: lo lanes idx (prefill first 3 rounds),
                    # hi lanes fp16 keys written by Act from score PSUM ----
                    packed = spool.tile([TQ, N], f32, tag="pk")
                    if t < 3:
                        nc.sync.dma_start(out=packed[:], in_=iota_d)
                    pk16 = packed[:].bitcast(i16).rearrange(
                        "p (w two) -> p w two", two=2)
                    hi = pk16[:, :, 1:2].bitcast(f16).rearrange("p w 1 -> p w")
                    for s in range(4):
                        ps = pss.tile([TQ, 1024], f32, tag="pssc")
                        nc.tensor.matmul(ps[:, 0:512], lhsT65[:, bass.ts(t, TQ)],
                                         rhs65[:, bass.ts(2 * s, 512)],
                                         start=True, stop=True)
                        nc.tensor.matmul(ps[:, 512:1024], lhsT65[:, bass.ts(t, TQ)],
                                         rhs65[:, bass.ts(2 * s + 1, 512)],
                                         start=True, stop=True)
                        nc.scalar.activation(
                            hi[:, bass.ts(s, 1024)],
                            ps[:], AF.Prelu, bias=biasq[:, t:t + 1], scale=1.0,
                            alpha=1.0)

                    # ---- top-16 merge (values carry indices) ----
                    cand = kpool.tile([TQ, NCH * 8], f32, tag="cand")
                    for c in range(NCH):
                        nc.vector.max(out=cand[:, bass.ts(c, 8)],
                                      in_=packed[:, bass.ts(c, CH)])
                    v8a = kpool.tile([TQ, 8], f32, tag="v8a")
                    nc.vector.max(out=v8a[:], in_=cand[:])
                    repl = kpool.tile([TQ, NCH * 8], f32, tag="repl")
                    nc.vector.match_replace(out=repl[:], in_to_replace=v8a[:],
                                            in_values=cand[:], imm_value=-1e30)
                    v8b = kpool.tile([TQ, 8], f32, tag="v8b")
                    nc.vector.max(out=v8b[:], in_=repl[:])

                    # extract lo-lane idx, replicate into 8 groups, transpose
                    idxf = kpool.tile([TQ, DM], i16, tag="idxf")
                    lo_a = v8a[:].bitcast(i16).rearrange(
                        "p (w two) -> p w two", two=2)[:, :, 0]
                    lo_b = v8b[:].bitcast(i16).rearrange(
                        "p (w two) -> p w two", two=2)[:, :, 0]
                    nc.vector.tensor_copy(out=idxf[:, 0:8], in_=lo_a)
                    nc.vector.tensor_copy(out=idxf[:, 8:16], in_=lo_b)
                    nc.vector.tensor_copy(out=idxf[:, 16:32], in_=idxf[:, 0:16])
                    nc.vector.tensor_copy(out=idxf[:, 32:64], in_=idxf[:, 0:32])
                    nc.vector.tensor_copy(out=idxf[:, 64:128], in_=idxf[:, 0:64])
                    idx16 = kpool.tile([TQ, TQ], i16, tag="idx16")
                    nc.sync.dma_start_transpose(out=idx16[:], in_=idxf[:])

                    # ---- gather [gk | v | gp] (transpose-mode limit: 512 idx/call) ----
                    gkv = []
                    for gh in range(4):
                        gt = gpool.tile([DM, 3, 512], f16, tag=f"gkv{gh}")
                        nc.gpsimd.dma_gather(
                            out_ap=gt[:], in_ap=table[:],
                            idxs_ap=idx16[:, bass.ts(gh, 32)],
                            num_idxs=512, num_idxs_reg=512, elem_size=ROW,
                            transpose=True)
                        gkv.append(gt)

                    # ---- pe MLP ----  h1 = prelu(gp_n + bd1 - gp_j)
                    zp = psp.tile([DM, PAIR], f32, tag="zpair")
                    for hh in range(4):
                        nc.tensor.matmul(zp[:, bass.ts(hh, 512)], gpT[:, bass.ts(t, TQ)],
                                         emat[:, bass.ts(hh, 512)], start=True, stop=False)
                    for hh in range(4):
                        nc.tensor.matmul(zp[:, bass.ts(hh, 512)], negi[:],
                                         gkv[hh][:, 2, :], start=False, stop=True)
                    h1 = prpool.tile([DM, PAIR], bft, tag="h1")
                    nc.scalar.activation(h1[:], zp[:], AF.Prelu, bias=bd1[:],
                                         scale=1.0, alpha=0.2)
                    # pe = prelu(wd2 . h1 + bd2); bd2 via rank-1 matmul
                    zp = psp.tile([DM, PAIR], f32, tag="zpair")
                    for hh in range(4):
                        sl = bass.ts(hh, 512)
                        nc.tensor.matmul(zp[:, sl], wd2t[:], h1[:, sl],
                                         start=True, stop=True)
                    pe = prpool.tile([DM, PAIR], bft, tag="pe")
                    nc.scalar.activation(pe[:], zp[:], AF.Prelu, bias=bd2[:],
                                         scale=1.0, alpha=0.2)

                    # ---- attention MLP ----  a1 = prelu(gq_n + bg1 - gk_j + wg1.pe)
                    zp = psp.tile([DM, PAIR], f32, tag="zpair")
                    for hh in range(4):
                        nc.tensor.matmul(zp[:, bass.ts(hh, 512)], gqT[:, bass.ts(t, TQ)],
                                         emat[:, bass.ts(hh, 512)], start=True, stop=False)
                    for hh in range(4):
                        nc.tensor.matmul(zp[:, bass.ts(hh, 512)], negi[:],
                                         gkv[hh][:, 0, :], start=False, stop=False)
                    for hh in range(4):
                        nc.tensor.matmul(zp[:, bass.ts(hh, 512)], wg1t[:],
                                         pe[:, bass.ts(hh, 512)], start=False, stop=True)
                    a1 = prpool.tile([DM, PAIR], bft, tag="a1")
                    nc.scalar.activation(a1[:], zp[:], AF.Prelu, bias=bg1[:],
                                         scale=1.0, alpha=0.2)
                    # a2 = prelu(wg2 . a1 + bg2); bg2 via rank-1 matmul
                    zp = psp.tile([DM, PAIR], f32, tag="zpair")
                    for hh in range(4):
                        sl = bass.ts(hh, 512)
                        nc.tensor.matmul(zp[:, sl], wg2t[:], a1[:, sl],
                                         start=True, stop=True)
                    a2 = prpool.tile([DM, PAIR], bft, tag="a2")
                    nc.scalar.activation(a2[:], zp[:], AF.Prelu, bias=bg2[:],
                                         scale=1.0, alpha=0.2)
                    ee = prpool.tile([DM, PAIR], bft, tag="ee")
                    nc.scalar.activation(ee[:], a2[:], AF.Exp, bias=0.0,
                                         scale=1.0 / 64.0)

                    # ---- softmax-normalized weighted sum ----
                    # ssum tree: 16 -> 8 -> 4 -> 2 -> 1 strided pairwise adds
                    ee4 = ee[:].rearrange("p (q two k) -> p q two k", two=2, k=8)
                    s8 = kpool.tile([DM, TQ * 8], bft, tag="s8")
                    nc.vector.tensor_add(s8[:].rearrange("p (q k) -> p q k", k=8),
                                         ee4[:, :, 0], ee4[:, :, 1])
                    s83 = s8[:].rearrange("p (q two k) -> p q two k", two=2, k=4)
                    s4 = kpool.tile([DM, TQ * 4], bft, tag="s4")
                    nc.vector.tensor_add(s4[:].rearrange("p (q k) -> p q k", k=4),
                                         s83[:, :, 0], s83[:, :, 1])
                    ssum = kpool.tile([DM, TQ], f32, tag="ssum")
                    nc.vector.reduce_sum(out=ssum[:],
                                         in_=s4[:].rearrange("p (q k) -> p q k", k=4),
                                         axis=AX.X)
                    rrec = kpool.tile([DM, TQ], f32, tag="rrec")
                    nc.vector.reciprocal(rrec[:], ssum[:])

                    ww = prpool.tile([DM, PAIR], bft, tag="ww")
                    for hh in range(4):
                        nc.vector.tensor_add(ww[:, bass.ts(hh, 512)],
                                             gkv[hh][:, 1, :],
                                             pe[:, bass.ts(hh, 512)])
                    nc.vector.tensor_mul(ww[:], ee[:], ww[:])
                    uu4 = ww[:].rearrange("p (q two k) -> p q two k", two=2, k=8)
                    u8 = kpool.tile([DM, TQ * 8], bft, tag="u8")
                    nc.vector.tensor_add(u8[:].rearrange("p (q k) -> p q k", k=8),
                                         uu4[:, :, 0], uu4[:, :, 1])
                    u83 = u8[:].rearrange("p (q two k) -> p q two k", two=2, k=4)
                    u4 = kpool.tile([DM, TQ * 4], bft, tag="u4")
                    nc.vector.tensor_add(u4[:].rearrange("p (q k) -> p q k", k=4),
                                         u83[:, :, 0], u83[:, :, 1])
                    ru = kpool.tile([DM, TQ], f32, tag="ru")
                    nc.vector.reduce_sum(out=ru[:],
                                         in_=u4[:].rearrange("p (q k) -> p q k", k=4),
                                         axis=AX.X)
                    nc.vector.tensor_mul(res_all[:, bass.ts(t, TQ)], ru[:], rrec[:])

            # ---------------- Phase C: output ----------------
            with (
                tc.tile_pool(name="outp", bufs=2) as opool,
                tc.tile_pool(name="ps_o", bufs=2, space="PSUM") as pso,
            ):
                own_f = opool.tile([DP, NQ], f32, tag="ownf")
                nc.sync.dma_start(out=own_f[:], in_=feats_own_f32)
                o1 = opool.tile([DP, NQ], f32, tag="o1")
                for s in range(4):
                    ps = pso.tile([DP, 512], f32, tag="pso")
                    nc.tensor.matmul(ps[:], w2t[:], res_all[:, bass.ts(s, 512)],
                                     start=True, stop=True)
                    nc.scalar.activation(o1[:, bass.ts(s, 512)], ps[:], AF.Prelu,
                                         bias=b2[:], scale=1.0, alpha=0.2)
                o2 = opool.tile([DP, NQ], f32, tag="o2")
                nc.vector.tensor_add(o2[:], o1[:], own_f[:])
                nc.sync.dma_start(out=out_d, in_=o2[:])

    nc.compile()
    return nc


def _host_prep(inputs):
    """Fold BN into weights, build per-core input maps."""
    s1, b1 = _fold_bn(np.asarray(inputs["bn1"]))
    sd1, bd1 = _fold_bn(np.asarray(inputs["bnd1"]))
    sd2, bd2 = _fold_bn(np.asarray(inputs["bnd2"]))
    sg1, bg1 = _fold_bn(np.asarray(inputs["bng1"]))
    sg2, bg2 = _fold_bn(np.asarray(inputs["bng2"]))
    s2, b2 = _fold_bn(np.asarray(inputs["bn2"]))
    W1f = np.asarray(inputs["W1"]) * s1[:, None]
    Wd1f = np.asarray(inputs["Wd1"]) * sd1[:, None]
    Wd2f = np.asarray(inputs["Wd2"]) * sd2[:, None]
    Wg1f = np.asarray(inputs["Wg1"]) * sg1[:, None]
    Wg2f = np.asarray(inputs["Wg2"]) * sg2[:, None]
    W2f = np.asarray(inputs["W2"]) * s2[:, None]
    Wg1k = (Wg1f @ np.asarray(inputs["Wk"])).astype(np.float32)
    Wg1q = (Wg1f @ np.asarray(inputs["Wq"])).astype(np.float32)
    Wv = np.asarray(inputs["Wv"], np.float32)

    E = np.zeros((TQ, PAIR), np.float32)
    for q in range(TQ):
        E[q, q * K:(q + 1) * K] = 1.0

    iota = np.broadcast_to(
        np.arange(N, dtype=np.uint32), (TQ, N)).astype(np.uint32)
    iota_f32 = iota.view(np.float32).copy()

    com = {
        "W1fT": np.ascontiguousarray(W1f.T, dtype=bf16),
        "WgkvT": np.ascontiguousarray(
            np.concatenate([Wg1k.T, Wv.T], axis=1), dtype=bf16),
        "Wg1qT": np.ascontiguousarray(Wg1q.T, dtype=bf16),
        "Wd1fT": np.ascontiguousarray(Wd1f.T, dtype=bf16),
        "Wd2fT": np.ascontiguousarray(Wd2f.T, dtype=bf16),
        "Wg1fT": np.ascontiguousarray(Wg1f.T, dtype=bf16),
        "Wg2fT": np.ascontiguousarray(Wg2f.T, dtype=bf16),
        "W2fT": np.ascontiguousarray(W2f.T, dtype=bf16),
        "E": E.astype(bf16),
        "negI": (-np.eye(DM)).astype(np.float16),
        "ident": np.eye(DM, dtype=np.float32),
        "ones1": np.ones((1, PAIR), dtype=bf16),
        "bd2r": np.ascontiguousarray(bd2.reshape(1, DM), dtype=bf16),
        "bg2r": np.ascontiguousarray(bg2.reshape(1, DM), dtype=bf16),
        "iotapk": iota_f32,
        "b1": b1.reshape(DM, 1),
        "bd1": bd1.reshape(DM, 1),
        "bd2": bd2.reshape(DM, 1),
        "bg1": bg1.reshape(DM, 1),
        "bg2": bg2.reshape(DM, 1),
        "b2": b2.reshape(DP, 1),
    }

    feats = np.asarray(inputs["feats"], np.float32)
    pos = np.asarray(inputs["pos"], np.float32)
    in_maps = []
    for c in range(8):
        b, h = c // 2, c % 2
        n0 = h * NQ
        fb = feats[b]
        sq = -0.5 * (fb.astype(np.float64) ** 2).sum(axis=0)
        rhs65 = np.empty((DP + 1, N), np.float32)
        rhs65[0:DP] = fb
        rhs65[DP] = sq.astype(np.float32)
        l65 = np.empty((DP + 1, NQ), np.float32)
        l65[0:DP] = fb[:, n0:n0 + NQ]
        l65[DP] = 1.0
        biasq = (KEY_C + sq[n0:n0 + NQ]).astype(np.float32).reshape(NT, TQ).T
        m = dict(com)
        m["rhs65"] = rhs65
        m["lhsT65"] = l65
        m["feats_own"] = np.ascontiguousarray(fb[:, n0:n0 + NQ])
        m["fb_own"] = np.ascontiguousarray(fb[:, n0:n0 + NQ], dtype=bf16)
        m["feats_bf"] = np.ascontiguousarray(fb, dtype=bf16)
        m["pos_bf"] = np.ascontiguousarray(pos[b], dtype=bf16)
        m["pos_own"] = np.ascontiguousarray(pos[b][:, n0:n0 + NQ], dtype=bf16)
        m["biasq"] = np.ascontiguousarray(biasq)
        in_maps.append(m)
    return in_maps


def kernel(**inputs):
    from concourse.bass_utils import run_bass_kernel_spmd

    if "nc" not in _CACHE:
        _CACHE["nc"] = _build_bass()
    nc = _CACHE["nc"]
    in_maps = _host_prep(inputs)
    r = run_bass_kernel_spmd(nc, in_maps, core_ids=list(range(8)),
                             **_CACHE.get("run_kwargs", {}))
    _CACHE["last_result"] = r
    out = np.empty((B, DP, N), np.float32)
    for c in range(8):
        b, h = c // 2, c % 2
        out[b][:, h * NQ:(h + 1) * NQ] = r.results[c]["out"]
    return out


# revision 15
# speedup vs baseline: 1.0197x; 1.0197x over previous
"""DGT block (dynamic graph transformer) Bass kernel for Trainium2.

Sharding: 8 cores = 4 batches x 2 query-halves. Each core handles one
batch's feats/pos and one half (2048) of the queries.

v2 vs baseline:
  - kNN scores via float32r matmul (1 cyc/row on PE vs fp32's 4).
  - top-16 without MaxIndex full-row scans: the PSUM->SBUF score copy on
    the Activation engine writes fp16 keys (score - self_score + 8, so
    the top keys sit in (0,8] with ~2^-11 relative quantum) into the hi
    16-bit lanes of an f32 "packed" tile whose lo lanes are pre-filled
    with the column index. Positive-float ordering == lexicographic
    (key, idx), so chunked max8 + merge yields values that carry their
    own indices; extraction is a strided 16-bit copy.
  - softmax sums as strided pairwise-add trees in DVE 2x (16-bit) mode.
  - h1/pe prelus on gpsimd (Pool), a1/a2 via Act; biases folded into the
    gq/gp precompute (h1, a1) or rank-1 bias matmuls on PE (pe, a2).
"""

import numpy as np
import ml_dtypes

B, N, K, DP, DM, EPS = 4, 4096, 16, 64, 128, 1e-5
NQ = N // 2            # queries per core
TQ = 128               # queries per tile
NT = NQ // TQ          # tiles per core (16)
PAIR = TQ * K          # pairs per tile (2048)
CH = 256               # top-k scan chunk size
NCH = N // CH          # 16 chunks
ROW = 3 * DM           # gather-table row elems (f16): [gk(128) | v(128) | gp(128)]
KEY_C = 8.0            # key offset: top keys in (0, 8]

_CACHE = {}

bf16 = ml_dtypes.bfloat16


def _fold_bn(p):
    g, be, m, v = p.astype(np.float64)
    s = g / np.sqrt(v + EPS)
    return (s).astype(np.float32), (be - m * s).astype(np.float32)


def _build_bass():
    import concourse.bass as bass
    import concourse.mybir as mybir
    import concourse.bacc as bacc
    from concourse.tile import TileContext

    dt = mybir.dt
    AF = mybir.ActivationFunctionType
    ALU = mybir.AluOpType
    AX = mybir.AxisListType

    nc = bacc.Bacc("TRN2", target_bir_lowering=False, debug=False, num_devices=8)

    # ---- I/O ----
    def inp(name, shape, dtype):
        return nc.dram_tensor(name, list(shape), dtype, kind="ExternalInput").ap()

    rhs65_d = inp("rhs65", (DP + 1, N), dt.float32r)   # [feats; -0.5*||f||^2]
    lhsT65_d = inp("lhsT65", (DP + 1, NQ), dt.float32r)  # [feats_own; 1]
    feats_own_f32 = inp("feats_own", (DP, NQ), dt.float32)
    fb_own_d = inp("fb_own", (DP, NQ), dt.bfloat16)
    feats_bf = inp("feats_bf", (DP, N), dt.bfloat16)
    pos_bf = inp("pos_bf", (3, N), dt.bfloat16)
    pos_own = inp("pos_own", (3, NQ), dt.bfloat16)
    biasq_d = inp("biasq", (TQ, NT), dt.float32)       # 8 - 0.5||f_q||^2 per tile col
    iota_d = inp("iotapk", (TQ, N), dt.float32)        # f32 bit pattern = col idx
    w1t_d = inp("W1fT", (DP, DM), dt.bfloat16)
    wkvt_d = inp("WgkvT", (DM, 2 * DM), dt.bfloat16)
    wqt_d = inp("Wg1qT", (DM, DM), dt.bfloat16)
    wd1t_d = inp("Wd1fT", (3, DM), dt.bfloat16)
    wd2t_d = inp("Wd2fT", (DM, DM), dt.bfloat16)
    wg1t_d = inp("Wg1fT", (DM, DM), dt.bfloat16)
    wg2t_d = inp("Wg2fT", (DM, DM), dt.bfloat16)
    w2t_d = inp("W2fT", (DM, DP), dt.bfloat16)
    e_d = inp("E", (TQ, PAIR), dt.bfloat16)
    negi_d = inp("negI", (DM, DM), dt.float16)
    ident_d = inp("ident", (DM, DM), dt.float32)
    ones1_d = inp("ones1", (1, PAIR), dt.bfloat16)     # rank-1 bias rhs
    bd2r_d = inp("bd2r", (1, DM), dt.bfloat16)         # bias rows (lhsT) for rank-1
    bg2r_d = inp("bg2r", (1, DM), dt.bfloat16)
    b1_d = inp("b1", (DM, 1), dt.float32)
    bd1_d = inp("bd1", (DM, 1), dt.float32)
    bd2_d = inp("bd2", (DM, 1), dt.float32)
    bg1_d = inp("bg1", (DM, 1), dt.float32)
    bg2_d = inp("bg2", (DM, 1), dt.float32)
    b2_d = inp("b2", (DP, 1), dt.float32)

    out_d = nc.dram_tensor("out", [DP, NQ], dt.float32, kind="ExternalOutput").ap()

    f32, f16, bft, i16, u16 = dt.float32, dt.float16, dt.bfloat16, dt.int16, dt.uint16

    with TileContext(nc) as tc:
        with (
            tc.tile_pool(name="const", bufs=1) as cpool,
            tc.tile_pool(name="persist", bufs=1) as ppool,
            tc.tile_pool(name="dram", bufs=1, space="DRAM") as dpool,
        ):
            # persistent constants
            w1t = cpool.tile_from(w1t_d)
            wkvt = cpool.tile_from(wkvt_d)
            wqt = cpool.tile_from(wqt_d)
            wd1t = cpool.tile_from(wd1t_d)
            wd2t = cpool.tile_from(wd2t_d)
            wg1t = cpool.tile_from(wg1t_d)
            wg2t = cpool.tile_from(wg2t_d)
            w2t = cpool.tile_from(w2t_d)
            emat = cpool.tile_from(e_d)
            negi = cpool.tile_from(negi_d)
            ident = cpool.tile_from(ident_d)
            ones1 = cpool.tile_from(ones1_d)
            bd2r = cpool.tile_from(bd2r_d)
            bg2r = cpool.tile_from(bg2r_d)
            biasq = cpool.tile_from(biasq_d)
            b1 = cpool.tile_from(b1_d)
            bd1 = cpool.tile_from(bd1_d)
            bd2 = cpool.tile_from(bd2_d)
            bg1 = cpool.tile_from(bg1_d)
            bg2 = cpool.tile_from(bg2_d)
            b2 = cpool.tile_from(b2_d)

            # persistent working tensors
            rhs65 = ppool.tile_from(rhs65_d)
            lhsT65 = ppool.tile_from(lhsT65_d)
            gqT = ppool.tile([TQ, NQ], bft)   # (q, m) blocks per tile, + bg1
            gpT = ppool.tile([TQ, NQ], bft)   # + bd1
            res_all = ppool.tile([DM, NQ], bft)
            table = dpool.tile([N, ROW], f16)

            # ---------------- Phase A: setup ----------------
            with (
                tc.tile_pool(name="setupA", bufs=2) as apool,
                tc.tile_pool(name="xpool", bufs=1) as xpool,
                tc.tile_pool(name="ps_a", bufs=2, space="PSUM") as psa,
                tc.tile_pool(name="ps_b", bufs=2, space="PSUM") as psb,
            ):
                fbt = xpool.tile([DP, N], bft)
                nc.sync.dma_start(out=fbt[:], in_=feats_bf)
                post = xpool.tile([3, N], bft)
                nc.sync.dma_start(out=post[:], in_=pos_bf)
                poso = xpool.tile([3, NQ], bft)
                nc.sync.dma_start(out=poso[:], in_=pos_own)
                fob = xpool.tile([DP, NQ], bft)
                nc.sync.dma_start(out=fob[:], in_=fb_own_d)

                xfull = xpool.tile([DM, N], bft)
                for s in range(8):
                    ps = psa.tile([DM, 512], f32, tag="psx")
                    nc.tensor.matmul(ps[:], w1t[:], fbt[:, bass.ts(s, 512)],
                                     start=True, stop=True)
                    nc.scalar.activation(xfull[:, bass.ts(s, 512)], ps[:],
                                         AF.Prelu, bias=b1[:], scale=1.0, alpha=0.2)
                xob = xpool.tile([DM, NQ], bft)
                for s in range(4):
                    ps = psa.tile([DM, 512], f32, tag="psx")
                    nc.tensor.matmul(ps[:], w1t[:], fob[:, bass.ts(s, 512)],
                                     start=True, stop=True)
                    nc.scalar.activation(xob[:, bass.ts(s, 512)], ps[:],
                                         AF.Prelu, bias=b1[:], scale=1.0, alpha=0.2)

                # gather table: rows [gk | v | gp] in f16
                for c in range(32):
                    pkv = psa.tile([TQ, 2 * DM], f32, tag="pskv")
                    nc.tensor.matmul(pkv[:], xfull[:, bass.ts(c, TQ)], wkvt[:],
                                     start=True, stop=True)
                    pgp = psb.tile([TQ, DM], f32, tag="psgp")
                    nc.tensor.matmul(pgp[:], post[:, bass.ts(c, TQ)], wd1t[:],
                                     start=True, stop=True)
                    stg = apool.tile([TQ, ROW], f16, tag="stg")
                    eng = nc.vector if (c % 2 == 0) else nc.scalar
                    if eng is nc.vector:
                        nc.vector.tensor_copy(out=stg[:, 0:2 * DM], in_=pkv[:])
                        nc.vector.tensor_copy(out=stg[:, 2 * DM:ROW], in_=pgp[:])
                    else:
                        nc.scalar.activation(stg[:, 0:2 * DM], pkv[:], AF.Copy)
                        nc.scalar.activation(stg[:, 2 * DM:ROW], pgp[:], AF.Copy)
                    nc.sync.dma_start(out=table[bass.ts(c, TQ), :], in_=stg[:])

                # gqT / gpT for own queries, with the h1/a1 biases folded in
                # via the PSUM->SBUF copy.
                for c in range(NT):
                    pq = psb.tile([TQ, DM], f32, tag="psgq")
                    nc.tensor.matmul(pq[:], xob[:, bass.ts(c, TQ)],
                                     wqt[:], start=True, stop=True)
                    pp = psb.tile([TQ, DM], f32, tag="psgq")
                    nc.tensor.matmul(pp[:], poso[:, bass.ts(c, TQ)], wd1t[:],
                                     start=True, stop=True)
                    if c % 2 == 0:
                        nc.vector.tensor_copy(out=gqT[:, bass.ts(c, DM)], in_=pq[:])
                        nc.vector.tensor_copy(out=gpT[:, bass.ts(c, DM)], in_=pp[:])
                    else:
                        nc.scalar.activation(gqT[:, bass.ts(c, DM)], pq[:], AF.Copy)
                        nc.scalar.activation(gpT[:, bass.ts(c, DM)], pp[:], AF.Copy)

            # ---------------- Phase B: per query tile ----------------
            with (
                tc.tile_pool(name="score", bufs=3) as spool,
                tc.tile_pool(name="gath", bufs=3) as gpool,
                tc.tile_pool(name="pair", bufs=2) as prpool,
                tc.tile_pool(name="topk", bufs=3) as kpool,
                tc.tile_pool(name="ps_s", bufs=2, space="PSUM") as pss,
                tc.tile_pool(name="ps_pair", bufs=1, space="PSUM") as psp,
            ):
                for t in range(NT):
                    # ---- packed tile: lo lanes idx (prefill first 3 rounds),
                    # hi lanes fp16 keys written by Act from score PSUM ----
                    packed = spool.tile([TQ, N], f32, tag="pk")
                    if t < 3:
                        nc.sync.dma_start(out=packed[:], in_=iota_d)
                    pk16 = packed[:].bitcast(i16).rearrange(
                        "p (w two) -> p w two", two=2)
                    hi = pk16[:, :, 1:2].bitcast(f16).rearrange("p w 1 -> p w")
                    for s in range(4):
                        ps = pss.tile([TQ, 1024], f32, tag="pssc")
                        nc.tensor.matmul(ps[:, 0:512], lhsT65[:, bass.ts(t, TQ)],
                                         rhs65[:, bass.ts(2 * s, 512)],
                                         start=True, stop=True)
                        nc.tensor.matmul(ps[:, 512:1024], lhsT65[:, bass.ts(t, TQ)],
                                         rhs65[:, bass.ts(2 * s + 1, 512)],
                                         start=True, stop=True)
                        nc.scalar.activation(
                            hi[:, bass.ts(s, 1024)],
                            ps[:], AF.Prelu, bias=biasq[:, t:t + 1], scale=1.0,
                            alpha=1.0)

                    # ---- top-16 merge (values carry indices) ----
                    cand = kpool.tile([TQ, NCH * 8], f32, tag="cand")
                    for c in range(NCH):
                        nc.vector.max(out=cand[:, bass.ts(c, 8)],
                                      in_=packed[:, bass.ts(c, CH)])
                    v8a = kpool.tile([TQ, 8], f32, tag="v8a")
                    nc.vector.max(out=v8a[:], in_=cand[:])
                    repl = kpool.tile([TQ, NCH * 8], f32, tag="repl")
                    nc.vector.match_replace(out=repl[:], in_to_replace=v8a[:],
                                            in_values=cand[:], imm_value=-1e30)
                    v8b = kpool.tile([TQ, 8], f32, tag="v8b")
                    nc.vector.max(out=v8b[:], in_=repl[:])

                    # extract lo-lane idx, replicate into 8 groups, transpose
                    idxf = kpool.tile([TQ, DM], i16, tag="idxf")
                    lo_a = v8a[:].bitcast(i16).rearrange(
                        "p (w two) -> p w two", two=2)[:, :, 0]
                    lo_b = v8b[:].bitcast(i16).rearrange(
                        "p (w two) -> p w two", two=2)[:, :, 0]
                    nc.vector.tensor_copy(out=idxf[:, 0:8], in_=lo_a)
                    nc.vector.tensor_copy(out=idxf[:, 8:16], in_=lo_b)
                    nc.vector.tensor_copy(out=idxf[:, 16:32], in_=idxf[:, 0:16])
                    nc.vector.tensor_copy(out=idxf[:, 32:64], in_=idxf[:, 0:32])
                    nc.vector.tensor_copy(out=idxf[:, 64:128], in_=idxf[:, 0:64])
                    idx16 = kpool.tile([TQ, TQ], i16, tag="idx16")
                    nc.sync.dma_start_transpose(out=idx16[:], in_=idxf[:])

                    # ---- gather [gk | v | gp] (transpose-mode limit: 512 idx/call) ----
                    gkv = []
                    for gh in range(4):
                        gt = gpool.tile([DM, 3, 512], f16, tag=f"gkv{gh}")
                        nc.gpsimd.dma_gather(
                            out_ap=gt[:], in_ap=table[:],
                            idxs_ap=idx16[:, bass.ts(gh, 32)],
                            num_idxs=512, num_idxs_reg=512, elem_size=ROW,
                            transpose=True)
                        gkv.append(gt)

                    # ---- pe MLP ----  h1 = prelu(gp_n + bd1 - gp_j)
                    zp = psp.tile([DM, PAIR], f32, tag="zpair")
                    for hh in range(4):
                        nc.tensor.matmul(zp[:, bass.ts(hh, 512)], gpT[:, bass.ts(t, TQ)],
                                         emat[:, bass.ts(hh, 512)], start=True, stop=False)
                    for hh in range(4):
                        nc.tensor.matmul(zp[:, bass.ts(hh, 512)], negi[:],
                                         gkv[hh][:, 2, :], start=False, stop=True)
                    h1 = prpool.tile([DM, PAIR], bft, tag="h1")
                    nc.scalar.activation(h1[:], zp[:], AF.Prelu, bias=bd1[:],
                                         scale=1.0, alpha=0.2)
                    # pe = prelu(wd2 . h1 + bd2); bd2 via rank-1 matmul
                    zp = psp.tile([DM, PAIR], f32, tag="zpair")
                    for hh in range(4):
                        sl = bass.ts(hh, 512)
                        nc.tensor.matmul(zp[:, sl], wd2t[:], h1[:, sl],
                                         start=True, stop=True)
                    pe = prpool.tile([DM, PAIR], bft, tag="pe")
                    nc.scalar.activation(pe[:], zp[:], AF.Prelu, bias=bd2[:],
                                         scale=1.0, alpha=0.2)

                    # ---- attention MLP ----  a1 = prelu(gq_n + bg1 - gk_j + wg1.pe)
                    zp = psp.tile([DM, PAIR], f32, tag="zpair")
                    for hh in range(4):
                        nc.tensor.matmul(zp[:, bass.ts(hh, 512)], gqT[:, bass.ts(t, TQ)],
                                         emat[:, bass.ts(hh, 512)], start=True, stop=False)
                    for hh in range(4):
                        nc.tensor.matmul(zp[:, bass.ts(hh, 512)], negi[:],
                                         gkv[hh][:, 0, :], start=False, stop=False)
                    for hh in range(4):
                        nc.tensor.matmul(zp[:, bass.ts(hh, 512)], wg1t[:],
                                         pe[:, bass.ts(hh, 512)], start=False, stop=True)
                    a1 = prpool.tile([DM, PAIR], bft, tag="a1")
                    nc.scalar.activation(a1[:], zp[:], AF.Prelu, bias=bg1[:],
                                         scale=1.0, alpha=0.2)
                    # a2 = prelu(wg2 . a1 + bg2); bg2 via rank-1 matmul
                    zp = psp.tile([DM, PAIR], f32, tag="zpair")
                    for hh in range(4):
                        sl = bass.ts(hh, 512)
                        nc.tensor.matmul(zp[:, sl], wg2t[:], a1[:, sl],
                                         start=True, stop=True)
                    a2 = prpool.tile([DM, PAIR], bft, tag="a2")
                    nc.scalar.activation(a2[:], zp[:], AF.Prelu, bias=bg2[:],
                                         scale=1.0, alpha=0.2)
                    ee = prpool.tile([DM, PAIR], bft, tag="ee")
                    nc.scalar.activation(ee[:], a2[:], AF.Exp, bias=0.0,
                                         scale=1.0 / 64.0)

                    # ---- softmax-normalized weighted sum ----
                    # ssum tree: 16 -> 8 -> 4 -> 2 -> 1 strided pairwise adds
                    ee4 = ee[:].rearrange("p (q two k) -> p q two k", two=2, k=8)
                    s8 = kpool.tile([DM, TQ * 8], bft, tag="s8")
                    nc.vector.tensor_add(s8[:].rearrange("p (q k) -> p q k", k=8),
                                         ee4[:, :, 0], ee4[:, :, 1])
                    s83 = s8[:].rearrange("p (q two k) -> p q two k", two=2, k=4)
                    s4 = kpool.tile([DM, TQ * 4], bft, tag="s4")
                    nc.vector.tensor_add(s4[:].rearrange("p (q k) -> p q k", k=4),
                                         s83[:, :, 0], s83[:, :, 1])
                    ssum = kpool.tile([DM, TQ], f32, tag="ssum")
                    nc.vector.reduce_sum(out=ssum[:],
                                         in_=s4[:].rearrange("p (q k) -> p q k", k=4),
                                         axis=AX.X)
                    rrec = kpool.tile([DM, TQ], f32, tag="rrec")
                    nc.vector.reciprocal(rrec[:], ssum[:])

                    ww = h1
                    for hh in range(4):
                        nc.vector.tensor_add(ww[:, bass.ts(hh, 512)],
                                             gkv[hh][:, 1, :],
                                             pe[:, bass.ts(hh, 512)])
                    nc.vector.tensor_mul(ww[:], ee[:], ww[:])
                    uu4 = ww[:].rearrange("p (q two k) -> p q two k", two=2, k=8)
                    u8 = kpool.tile([DM, TQ * 8], bft, tag="u8")
                    nc.vector.tensor_add(u8[:].rearrange("p (q k) -> p q k", k=8),
                                         uu4[:, :, 0], uu4[:, :, 1])
                    u83 = u8[:].rearrange("p (q two k) -> p q two k", two=2, k=4)
                    u4 = kpool.tile([DM, TQ * 4], bft, tag="u4")
                    nc.vector.tensor_add(u4[:].rearrange("p (q k) -> p q k", k=4),
                                         u83[:, :, 0], u83[:, :, 1])
                    ru = kpool.tile([DM, TQ], f32, tag="ru")
                    nc.vector.reduce_sum(out=ru[:],
                                         in_=u4[:].rearrange("p (q k) -> p q k", k=4),
                                         axis=AX.X)
                    nc.vector.tensor_mul(res_all[:, bass.ts(t, TQ)], ru[:], rrec[:])

            # ---------------- Phase C: output ----------------
            with (
                tc.tile_pool(name="outp", bufs=2) as opool,
                tc.tile_pool(name="ps_o", bufs=2, space="PSUM") as pso,
            ):
                own_f = opool.tile([DP, NQ], f32, tag="ownf")
                nc.sync.dma_start(out=own_f[:], in_=feats_own_f32)
                o1 = opool.tile([DP, NQ], f32, tag="o1")
                for s in range(4):
                    ps = pso.tile([DP, 512], f32, tag="pso")
                    nc.tensor.matmul(ps[:], w2t[:], res_all[:, bass.ts(s, 512)],
                                     start=True, stop=True)
                    nc.scalar.activation(o1[:, bass.ts(s, 512)], ps[:], AF.Prelu,
                                         bias=b2[:], scale=1.0, alpha=0.2)
                o2 = opool.tile([DP, NQ], f32, tag="o2")
                nc.vector.tensor_add(o2[:], o1[:], own_f[:])
                nc.sync.dma_start(out=out_d, in_=o2[:])

    nc.compile()
    return nc


def _host_prep(inputs):
    """Fold BN into weights, build per-core input maps."""
    s1, b1 = _fold_bn(np.asarray(inputs["bn1"]))
    sd1, bd1 = _fold_bn(np.asarray(inputs["bnd1"]))
    sd2, bd2 = _fold_bn(np.asarray(inputs["bnd2"]))
    sg1, bg1 = _fold_bn(np.asarray(inputs["bng1"]))
    sg2, bg2 = _fold_bn(np.asarray(inputs["bng2"]))
    s2, b2 = _fold_bn(np.asarray(inputs["bn2"]))
    W1f = np.asarray(inputs["W1"]) * s1[:, None]
    Wd1f = np.asarray(inputs["Wd1"]) * sd1[:, None]
    Wd2f = np.asarray(inputs["Wd2"]) * sd2[:, None]
    Wg1f = np.asarray(inputs["Wg1"]) * sg1[:, None]
    Wg2f = np.asarray(inputs["Wg2"]) * sg2[:, None]
    W2f = np.asarray(inputs["W2"]) * s2[:, None]
    Wg1k = (Wg1f @ np.asarray(inputs["Wk"])).astype(np.float32)
    Wg1q = (Wg1f @ np.asarray(inputs["Wq"])).astype(np.float32)
    Wv = np.asarray(inputs["Wv"], np.float32)

    E = np.zeros((TQ, PAIR), np.float32)
    for q in range(TQ):
        E[q, q * K:(q + 1) * K] = 1.0

    iota = np.broadcast_to(
        np.arange(N, dtype=np.uint32), (TQ, N)).astype(np.uint32)
    iota_f32 = iota.view(np.float32).copy()

    com = {
        "W1fT": np.ascontiguousarray(W1f.T, dtype=bf16),
        "WgkvT": np.ascontiguousarray(
            np.concatenate([Wg1k.T, Wv.T], axis=1), dtype=bf16),
        "Wg1qT": np.ascontiguousarray(Wg1q.T, dtype=bf16),
        "Wd1fT": np.ascontiguousarray(Wd1f.T, dtype=bf16),
        "Wd2fT": np.ascontiguousarray(Wd2f.T, dtype=bf16),
        "Wg1fT": np.ascontiguousarray(Wg1f.T, dtype=bf16),
        "Wg2fT": np.ascontiguousarray(Wg2f.T, dtype=bf16),
        "W2fT": np.ascontiguousarray(W2f.T, dtype=bf16),
        "E": E.astype(bf16),
        "negI": (-np.eye(DM)).astype(np.float16),
        "ident": np.eye(DM, dtype=np.float32),
        "ones1": np.ones((1, PAIR), dtype=bf16),
        "bd2r": np.ascontiguousarray(bd2.reshape(1, DM), dtype=bf16),
        "bg2r": np.ascontiguousarray(bg2.reshape(1, DM), dtype=bf16),
        "iotapk": iota_f32,
        "b1": b1.reshape(DM, 1),
        "bd1": bd1.reshape(DM, 1),
        "bd2": bd2.reshape(DM, 1),
        "bg1": bg1.reshape(DM, 1),
        "bg2": bg2.reshape(DM, 1),
        "b2": b2.reshape(DP, 1),
    }

    feats = np.asarray(inputs["feats"], np.float32)
    pos = np.asarray(inputs["pos"], np.float32)
    in_maps = []
    for c in range(8):
        b, h = c // 2, c % 2
        n0 = h * NQ
        fb = feats[b]
        sq = -0.5 * (fb.astype(np.float64) ** 2).sum(axis=0)
        rhs65 = np.empty((DP + 1, N), np.float32)
        rhs65[0:DP] = fb
        rhs65[DP] = sq.astype(np.float32)
        l65 = np.empty((DP + 1, NQ), np.float32)
        l65[0:DP] = fb[:, n0:n0 + NQ]
        l65[DP] = 1.0
        biasq = (KEY_C + sq[n0:n0 + NQ]).astype(np.float32).reshape(NT, TQ).T
        m = dict(com)
        m["rhs65"] = rhs65
        m["lhsT65"] = l65
        m["feats_own"] = np.ascontiguousarray(fb[:, n0:n0 + NQ])
        m["fb_own"] = np.ascontiguousarray(fb[:, n0:n0 + NQ], dtype=bf16)
        m["feats_bf"] = np.ascontiguousarray(fb, dtype=bf16)
        m["pos_bf"] = np.ascontiguousarray(pos[b], dtype=bf16)
        m["pos_own"] = np.ascontiguousarray(pos[b][:, n0:n0 + NQ], dtype=bf16)
        m["biasq"] = np.ascontiguousarray(biasq)
        in_maps.append(m)
    return in_maps


def kernel(**inputs):
    from concourse.bass_utils import run_bass_kernel_spmd

    if "nc" not in _CACHE:
        _CACHE["nc"] = _build_bass()
    nc = _CACHE["nc"]
    in_maps = _host_prep(inputs)
    r = run_bass_kernel_spmd(nc, in_maps, core_ids=list(range(8)),
                             **_CACHE.get("run_kwargs", {}))
    _CACHE["last_result"] = r
    out = np.empty((B, DP, N), np.float32)
    for c in range(8):
        b, h = c // 2, c % 2
        out[b][:, h * NQ:(h + 1) * NQ] = r.results[c]["out"]
    return out


# revision 16
# speedup vs baseline: 1.0694x; 1.0487x over previous
"""DGT block (dynamic graph transformer) Bass kernel for Trainium2.

Sharding: 8 cores = 4 batches x 2 query-halves. Each core handles one
batch's feats/pos and one half (2048) of the queries.

v2 vs baseline:
  - kNN scores via float32r matmul (1 cyc/row on PE vs fp32's 4).
  - top-16 without MaxIndex full-row scans: the PSUM->SBUF score copy on
    the Activation engine writes fp16 keys (score - self_score + 8, so
    the top keys sit in (0,8] with ~2^-11 relative quantum) into the hi
    16-bit lanes of an f32 "packed" tile whose lo lanes are pre-filled
    with the column index. Positive-float ordering == lexicographic
    (key, idx), so chunked max8 + merge yields values that carry their
    own indices; extraction is a strided 16-bit copy.
  - softmax sums as strided pairwise-add trees in DVE 2x (16-bit) mode.
  - h1/pe prelus on gpsimd (Pool), a1/a2 via Act; biases folded into the
    gq/gp precompute (h1, a1) or rank-1 bias matmuls on PE (pe, a2).
"""

import numpy as np
import ml_dtypes

B, N, K, DP, DM, EPS = 4, 4096, 16, 64, 128, 1e-5
NQ = N // 2            # queries per core
TQ = 128               # queries per tile
NT = NQ // TQ          # tiles per core (16)
PAIR = TQ * K          # pairs per tile (2048)
CH = 256               # top-k scan chunk size
NCH = N // CH          # 16 chunks
ROW = 3 * DM           # gather-table row elems (f16): [gk(128) | v(128) | gp(128)]
KEY_C = 8.0            # key offset: top keys in (0, 8]

_CACHE = {}

bf16 = ml_dtypes.bfloat16


def _fold_bn(p):
    g, be, m, v = p.astype(np.float64)
    s = g / np.sqrt(v + EPS)
    return (s).astype(np.float32), (be - m * s).astype(np.float32)


def _build_bass():
    import concourse.bass as bass
    import concourse.mybir as mybir
    import concourse.bacc as bacc
    from concourse.tile import TileContext

    dt = mybir.dt
    AF = mybir.ActivationFunctionType
    ALU = mybir.AluOpType
    AX = mybir.AxisListType

    nc = bacc.Bacc("TRN2", target_bir_lowering=False, debug=False, num_devices=8)

    # ---- I/O ----
    def inp(name, shape, dtype):
        return nc.dram_tensor(name, list(shape), dtype, kind="ExternalInput").ap()

    rhs65_d = inp("rhs65", (DP + 1, N), dt.float32r)   # [feats; -0.5*||f||^2]
    lhsT65_d = inp("lhsT65", (DP + 1, NQ), dt.float32r)  # [feats_own; 1]
    feats_own_f32 = inp("feats_own", (DP, NQ), dt.float32)
    fb_own_d = inp("fb_own", (DP, NQ), dt.bfloat16)
    feats_bf = inp("feats_bf", (DP, N), dt.bfloat16)
    pos_bf = inp("pos_bf", (3, N), dt.bfloat16)
    pos_own = inp("pos_own", (3, NQ), dt.bfloat16)
    biasq_d = inp("biasq", (TQ, NT), dt.float32)       # 8 - 0.5||f_q||^2 per tile col
    iota_d = inp("iotapk", (TQ, N), dt.float32)        # f32 bit pattern = col idx
    w1t_d = inp("W1fT", (DP, DM), dt.bfloat16)
    wkvt_d = inp("WgkvT", (DM, 2 * DM), dt.bfloat16)
    wqt_d = inp("Wg1qT", (DM, DM), dt.bfloat16)
    wd1t_d = inp("Wd1fT", (3, DM), dt.bfloat16)
    wd2t_d = inp("Wd2fT", (DM, DM), dt.bfloat16)
    wg1t_d = inp("Wg1fT", (DM, DM), dt.bfloat16)
    wg2t_d = inp("Wg2fT", (DM, DM), dt.bfloat16)
    w2t_d = inp("W2fT", (DM, DP), dt.bfloat16)
    e_d = inp("E", (TQ, PAIR), dt.bfloat16)
    negi_d = inp("negI", (DM, DM), dt.float16)
    ident_d = inp("ident", (DM, DM), dt.float32)
    ones1_d = inp("ones1", (1, PAIR), dt.bfloat16)     # rank-1 bias rhs
    bd2r_d = inp("bd2r", (1, DM), dt.bfloat16)         # bias rows (lhsT) for rank-1
    bg2r_d = inp("bg2r", (1, DM), dt.bfloat16)
    b1_d = inp("b1", (DM, 1), dt.float32)
    bd1_d = inp("bd1", (DM, 1), dt.float32)
    bd2_d = inp("bd2", (DM, 1), dt.float32)
    bg1_d = inp("bg1", (DM, 1), dt.float32)
    bg2_d = inp("bg2", (DM, 1), dt.float32)
    b2_d = inp("b2", (DP, 1), dt.float32)

    out_d = nc.dram_tensor("out", [DP, NQ], dt.float32, kind="ExternalOutput").ap()

    f32, f16, bft, i16, u16 = dt.float32, dt.float16, dt.bfloat16, dt.int16, dt.uint16

    with TileContext(nc) as tc:
        with (
            tc.tile_pool(name="const", bufs=1) as cpool,
            tc.tile_pool(name="persist", bufs=1) as ppool,
            tc.tile_pool(name="dram", bufs=1, space="DRAM") as dpool,
        ):
            # persistent constants
            w1t = cpool.tile_from(w1t_d)
            wkvt = cpool.tile_from(wkvt_d)
            wqt = cpool.tile_from(wqt_d)
            wd1t = cpool.tile_from(wd1t_d)
            wd2t = cpool.tile_from(wd2t_d)
            wg1t = cpool.tile_from(wg1t_d)
            wg2t = cpool.tile_from(wg2t_d)
            w2t = cpool.tile_from(w2t_d)
            emat = cpool.tile_from(e_d)
            negi = cpool.tile_from(negi_d)
            ident = cpool.tile_from(ident_d)
            ones1 = cpool.tile_from(ones1_d)
            bd2r = cpool.tile_from(bd2r_d)
            bg2r = cpool.tile_from(bg2r_d)
            biasq = cpool.tile_from(biasq_d)
            b1 = cpool.tile_from(b1_d)
            bd1 = cpool.tile_from(bd1_d)
            bd2 = cpool.tile_from(bd2_d)
            bg1 = cpool.tile_from(bg1_d)
            bg2 = cpool.tile_from(bg2_d)
            b2 = cpool.tile_from(b2_d)

            # persistent working tensors
            rhs65 = ppool.tile_from(rhs65_d)
            lhsT65 = ppool.tile_from(lhsT65_d)
            gqT = ppool.tile([TQ, NQ], bft)   # (q, m) blocks per tile, + bg1
            gpT = ppool.tile([TQ, NQ], bft)   # + bd1
            res_all = ppool.tile([DM, NQ], bft)
            table = dpool.tile([N, ROW], f16)

            # ---------------- Phase A: setup ----------------
            with (
                tc.tile_pool(name="setupA", bufs=2) as apool,
                tc.tile_pool(name="xpool", bufs=1) as xpool,
                tc.tile_pool(name="ps_a", bufs=2, space="PSUM") as psa,
                tc.tile_pool(name="ps_b", bufs=2, space="PSUM") as psb,
            ):
                fbt = xpool.tile([DP, N], bft)
                nc.sync.dma_start(out=fbt[:], in_=feats_bf)
                post = xpool.tile([3, N], bft)
                nc.sync.dma_start(out=post[:], in_=pos_bf)
                poso = xpool.tile([3, NQ], bft)
                nc.sync.dma_start(out=poso[:], in_=pos_own)
                fob = xpool.tile([DP, NQ], bft)
                nc.sync.dma_start(out=fob[:], in_=fb_own_d)

                xfull = xpool.tile([DM, N], bft)
                for s in range(8):
                    ps = psa.tile([DM, 512], f32, tag="psx")
                    nc.tensor.matmul(ps[:], w1t[:], fbt[:, bass.ts(s, 512)],
                                     start=True, stop=True)
                    nc.scalar.activation(xfull[:, bass.ts(s, 512)], ps[:],
                                         AF.Prelu, bias=b1[:], scale=1.0, alpha=0.2)
                xob = xpool.tile([DM, NQ], bft)
                for s in range(4):
                    ps = psa.tile([DM, 512], f32, tag="psx")
                    nc.tensor.matmul(ps[:], w1t[:], fob[:, bass.ts(s, 512)],
                                     start=True, stop=True)
                    nc.scalar.activation(xob[:, bass.ts(s, 512)], ps[:],
                                         AF.Prelu, bias=b1[:], scale=1.0, alpha=0.2)

                # gather table: rows [gk | v | gp] in f16
                for c in range(32):
                    pkv = psa.tile([TQ, 2 * DM], f32, tag="pskv")
                    nc.tensor.matmul(pkv[:], xfull[:, bass.ts(c, TQ)], wkvt[:],
                                     start=True, stop=True)
                    pgp = psb.tile([TQ, DM], f32, tag="psgp")
                    nc.tensor.matmul(pgp[:], post[:, bass.ts(c, TQ)], wd1t[:],
                                     start=True, stop=True)
                    stg = apool.tile([TQ, ROW], f16, tag="stg")
                    nc.vector.tensor_copy(out=stg[:, 0:2 * DM], in_=pkv[:])
                    nc.vector.tensor_copy(out=stg[:, 2 * DM:ROW], in_=pgp[:])
                    nc.sync.dma_start(out=table[bass.ts(c, TQ), :], in_=stg[:])

                # gqT / gpT for own queries, with the h1/a1 biases folded in
                # via the PSUM->SBUF copy.
                for c in range(NT):
                    pq = psb.tile([TQ, DM], f32, tag="psgq")
                    nc.tensor.matmul(pq[:], xob[:, bass.ts(c, TQ)],
                                     wqt[:], start=True, stop=True)
                    pp = psb.tile([TQ, DM], f32, tag="psgq")
                    nc.tensor.matmul(pp[:], poso[:, bass.ts(c, TQ)], wd1t[:],
                                     start=True, stop=True)
                    nc.scalar.activation(gqT[:, bass.ts(c, DM)], pq[:], AF.Copy)
                    nc.scalar.activation(gpT[:, bass.ts(c, DM)], pp[:], AF.Copy)

            # ---------------- Phase B: per query tile ----------------
            with (
                tc.tile_pool(name="score", bufs=3) as spool,
                tc.tile_pool(name="gath", bufs=3) as gpool,
                tc.tile_pool(name="pair", bufs=2) as prpool,
                tc.tile_pool(name="topk", bufs=3) as kpool,
                tc.tile_pool(name="ps_s", bufs=2, space="PSUM") as pss,
                tc.tile_pool(name="ps_pair", bufs=1, space="PSUM") as psp,
            ):
                for t in range(NT):
                    # ---- packed tile: lo lanes idx (prefill first 3 rounds),
                    # hi lanes fp16 keys written by Act from score PSUM ----
                    packed = spool.tile([TQ, N], f32, tag="pk")
                    if t < 3:
                        nc.sync.dma_start(out=packed[:], in_=iota_d)
                    pk16 = packed[:].bitcast(i16).rearrange(
                        "p (w two) -> p w two", two=2)
                    hi = pk16[:, :, 1:2].bitcast(f16).rearrange("p w 1 -> p w")
                    for s in range(4):
                        ps = pss.tile([TQ, 1024], f32, tag="pssc")
                        nc.tensor.matmul(ps[:, 0:512], lhsT65[:, bass.ts(t, TQ)],
                                         rhs65[:, bass.ts(2 * s, 512)],
                                         start=True, stop=True)
                        nc.tensor.matmul(ps[:, 512:1024], lhsT65[:, bass.ts(t, TQ)],
                                         rhs65[:, bass.ts(2 * s + 1, 512)],
                                         start=True, stop=True)
                        if s == 3:
                            nc.vector.tensor_scalar(
                                out=hi[:, bass.ts(s, 1024)], in0=ps[:],
                                scalar1=biasq[:, t:t + 1], scalar2=None,
                                op0=ALU.add)
                        else:
                            nc.scalar.activation(
                                hi[:, bass.ts(s, 1024)],
                                ps[:], AF.Prelu, bias=biasq[:, t:t + 1], scale=1.0,
                                alpha=1.0)

                    # ---- top-16 merge (values carry indices) ----
                    cand = kpool.tile([TQ, NCH * 8], f32, tag="cand")
                    for c in range(NCH):
                        nc.vector.max(out=cand[:, bass.ts(c, 8)],
                                      in_=packed[:, bass.ts(c, CH)])
                    v8a = kpool.tile([TQ, 8], f32, tag="v8a")
                    nc.vector.max(out=v8a[:], in_=cand[:])
                    repl = kpool.tile([TQ, NCH * 8], f32, tag="repl")
                    nc.vector.match_replace(out=repl[:], in_to_replace=v8a[:],
                                            in_values=cand[:], imm_value=-1e30)
                    v8b = kpool.tile([TQ, 8], f32, tag="v8b")
                    nc.vector.max(out=v8b[:], in_=repl[:])

                    # extract lo-lane idx, replicate into 8 groups, transpose
                    idxf = kpool.tile([TQ, DM], i16, tag="idxf")
                    lo_a = v8a[:].bitcast(i16).rearrange(
                        "p (w two) -> p w two", two=2)[:, :, 0]
                    lo_b = v8b[:].bitcast(i16).rearrange(
                        "p (w two) -> p w two", two=2)[:, :, 0]
                    nc.vector.tensor_copy(out=idxf[:, 0:8], in_=lo_a)
                    nc.vector.tensor_copy(out=idxf[:, 8:16], in_=lo_b)
                    nc.gpsimd.tensor_copy(out=idxf[:, 16:32], in_=idxf[:, 0:16])
                    nc.gpsimd.tensor_copy(out=idxf[:, 32:64], in_=idxf[:, 0:32])
                    nc.gpsimd.tensor_copy(out=idxf[:, 64:128], in_=idxf[:, 0:64])
                    idx16 = kpool.tile([TQ, TQ], i16, tag="idx16")
                    nc.sync.dma_start_transpose(out=idx16[:], in_=idxf[:])

                    # ---- gather [gk | v | gp] (transpose-mode limit: 512 idx/call) ----
                    gkv = []
                    for gh in range(4):
                        gt = gpool.tile([DM, 3, 512], f16, tag=f"gkv{gh}")
                        nc.gpsimd.dma_gather(
                            out_ap=gt[:], in_ap=table[:],
                            idxs_ap=idx16[:, bass.ts(gh, 32)],
                            num_idxs=512, num_idxs_reg=512, elem_size=ROW,
                            transpose=True)
                        gkv.append(gt)

                    # ---- pe MLP ----  h1 = prelu(gp_n + bd1 - gp_j)
                    zp = psp.tile([DM, PAIR], f32, tag="zpair")
                    for hh in range(4):
                        nc.tensor.matmul(zp[:, bass.ts(hh, 512)], gpT[:, bass.ts(t, TQ)],
                                         emat[:, bass.ts(hh, 512)], start=True, stop=False)
                    for hh in range(4):
                        nc.tensor.matmul(zp[:, bass.ts(hh, 512)], negi[:],
                                         gkv[hh][:, 2, :], start=False, stop=True)
                    h1 = prpool.tile([DM, PAIR], bft, tag="h1")
                    nc.scalar.activation(h1[:], zp[:], AF.Prelu, bias=bd1[:],
                                         scale=1.0, alpha=0.2)
                    # pe = prelu(wd2 . h1 + bd2); bd2 via rank-1 matmul
                    zp = psp.tile([DM, PAIR], f32, tag="zpair")
                    for hh in range(4):
                        sl = bass.ts(hh, 512)
                        nc.tensor.matmul(zp[:, sl], wd2t[:], h1[:, sl],
                                         start=True, stop=True)
                    pe = prpool.tile([DM, PAIR], bft, tag="pe")
                    nc.scalar.activation(pe[:], zp[:], AF.Prelu, bias=bd2[:],
                                         scale=1.0, alpha=0.2)

                    # ---- attention MLP ----  a1 = prelu(gq_n + bg1 - gk_j + wg1.pe)
                    zp = psp.tile([DM, PAIR], f32, tag="zpair")
                    for hh in range(4):
                        nc.tensor.matmul(zp[:, bass.ts(hh, 512)], gqT[:, bass.ts(t, TQ)],
                                         emat[:, bass.ts(hh, 512)], start=True, stop=False)
                    for hh in range(4):
                        nc.tensor.matmul(zp[:, bass.ts(hh, 512)], negi[:],
                                         gkv[hh][:, 0, :], start=False, stop=False)
                    for hh in range(4):
                        nc.tensor.matmul(zp[:, bass.ts(hh, 512)], wg1t[:],
                                         pe[:, bass.ts(hh, 512)], start=False, stop=True)
                    a1 = prpool.tile([DM, PAIR], bft, tag="a1")
                    nc.scalar.activation(a1[:], zp[:], AF.Prelu, bias=bg1[:],
                                         scale=1.0, alpha=0.2)
                    # a2 = prelu(wg2 . a1 + bg2); bg2 via rank-1 matmul
                    zp = psp.tile([DM, PAIR], f32, tag="zpair")
                    for hh in range(4):
                        sl = bass.ts(hh, 512)
                        nc.tensor.matmul(zp[:, sl], wg2t[:], a1[:, sl],
                                         start=True, stop=True)
                    a2 = prpool.tile([DM, PAIR], bft, tag="a2")
                    nc.scalar.activation(a2[:], zp[:], AF.Prelu, bias=bg2[:],
                                         scale=1.0, alpha=0.2)
                    ee = prpool.tile([DM, PAIR], bft, tag="ee")
                    nc.scalar.activation(ee[:], a2[:], AF.Exp, bias=0.0,
                                         scale=1.0 / 64.0)

                    # ---- softmax-normalized weighted sum ----
                    # ssum tree: 16 -> 8 -> 4 -> 2 -> 1 strided pairwise adds
                    ee4 = ee[:].rearrange("p (q two k) -> p q two k", two=2, k=8)
                    s8 = kpool.tile([DM, TQ * 8], bft, tag="s8")
                    nc.vector.tensor_add(s8[:].rearrange("p (q k) -> p q k", k=8),
                                         ee4[:, :, 0], ee4[:, :, 1])
                    s83 = s8[:].rearrange("p (q two k) -> p q two k", two=2, k=4)
                    s4 = kpool.tile([DM, TQ * 4], bft, tag="s4")
                    nc.vector.tensor_add(s4[:].rearrange("p (q k) -> p q k", k=4),
                                         s83[:, :, 0], s83[:, :, 1])
                    s43 = s4[:].rearrange("p (q two k) -> p q two k", two=2, k=2)
                    s2 = kpool.tile([DM, TQ * 2], bft, tag="s2")
                    nc.vector.tensor_add(s2[:].rearrange("p (q k) -> p q k", k=2),
                                         s43[:, :, 0], s43[:, :, 1])
                    s23 = s2[:].rearrange("p (q two) -> p q two", two=2)
                    ssum = kpool.tile([DM, TQ], f32, tag="ssum")
                    nc.vector.tensor_add(ssum[:], s23[:, :, 0], s23[:, :, 1])
                    rrec = kpool.tile([DM, TQ], f32, tag="rrec")
                    nc.vector.reciprocal(rrec[:], ssum[:])

                    ww = h1
                    for hh in range(4):
                        nc.gpsimd.tensor_add(ww[:, bass.ts(hh, 512)],
                                             gkv[hh][:, 1, :],
                                             pe[:, bass.ts(hh, 512)])
                    nc.vector.tensor_mul(ww[:], ee[:], ww[:])
                    uu4 = ww[:].rearrange("p (q two k) -> p q two k", two=2, k=8)
                    u8 = kpool.tile([DM, TQ * 8], bft, tag="u8")
                    nc.vector.tensor_add(u8[:].rearrange("p (q k) -> p q k", k=8),
                                         uu4[:, :, 0], uu4[:, :, 1])
                    u83 = u8[:].rearrange("p (q two k) -> p q two k", two=2, k=4)
                    u4 = kpool.tile([DM, TQ * 4], bft, tag="u4")
                    nc.vector.tensor_add(u4[:].rearrange("p (q k) -> p q k", k=4),
                                         u83[:, :, 0], u83[:, :, 1])
                    u43 = u4[:].rearrange("p (q two k) -> p q two k", two=2, k=2)
                    u2 = kpool.tile([DM, TQ * 2], bft, tag="u2")
                    nc.vector.tensor_add(u2[:].rearrange("p (q k) -> p q k", k=2),
                                         u43[:, :, 0], u43[:, :, 1])
                    u23 = u2[:].rearrange("p (q two) -> p q two", two=2)
                    ru = kpool.tile([DM, TQ], f32, tag="ru")
                    nc.vector.tensor_add(ru[:], u23[:, :, 0], u23[:, :, 1])
                    nc.vector.tensor_mul(res_all[:, bass.ts(t, TQ)], ru[:], rrec[:])

            # ---------------- Phase C: output ----------------
            with (
                tc.tile_pool(name="outp", bufs=2) as opool,
                tc.tile_pool(name="ps_o", bufs=2, space="PSUM") as pso,
            ):
                own_f = opool.tile([DP, NQ], f32, tag="ownf")
                nc.sync.dma_start(out=own_f[:], in_=feats_own_f32)
                o1 = opool.tile([DP, NQ], f32, tag="o1")
                for s in range(4):
                    ps = pso.tile([DP, 512], f32, tag="pso")
                    nc.tensor.matmul(ps[:], w2t[:], res_all[:, bass.ts(s, 512)],
                                     start=True, stop=True)
                    nc.scalar.activation(o1[:, bass.ts(s, 512)], ps[:], AF.Prelu,
                                         bias=b2[:], scale=1.0, alpha=0.2)
                o2 = opool.tile([DP, NQ], f32, tag="o2")
                nc.vector.tensor_add(o2[:], o1[:], own_f[:])
                nc.sync.dma_start(out=out_d, in_=o2[:])

    nc.compile()
    return nc


def _host_prep(inputs):
    """Fold BN into weights, build per-core input maps."""
    s1, b1 = _fold_bn(np.asarray(inputs["bn1"]))
    sd1, bd1 = _fold_bn(np.asarray(inputs["bnd1"]))
    sd2, bd2 = _fold_bn(np.asarray(inputs["bnd2"]))
    sg1, bg1 = _fold_bn(np.asarray(inputs["bng1"]))
    sg2, bg2 = _fold_bn(np.asarray(inputs["bng2"]))
    s2, b2 = _fold_bn(np.asarray(inputs["bn2"]))
    W1f = np.asarray(inputs["W1"]) * s1[:, None]
    Wd1f = np.asarray(inputs["Wd1"]) * sd1[:, None]
    Wd2f = np.asarray(inputs["Wd2"]) * sd2[:, None]
    Wg1f = np.asarray(inputs["Wg1"]) * sg1[:, None]
    Wg2f = np.asarray(inputs["Wg2"]) * sg2[:, None]
    W2f = np.asarray(inputs["W2"]) * s2[:, None]
    Wg1k = (Wg1f @ np.asarray(inputs["Wk"])).astype(np.float32)
    Wg1q = (Wg1f @ np.asarray(inputs["Wq"])).astype(np.float32)
    Wv = np.asarray(inputs["Wv"], np.float32)

    E = np.zeros((TQ, PAIR), np.float32)
    for q in range(TQ):
        E[q, q * K:(q + 1) * K] = 1.0

    iota = np.broadcast_to(
        np.arange(N, dtype=np.uint32), (TQ, N)).astype(np.uint32)
    iota_f32 = iota.view(np.float32).copy()

    com = {
        "W1fT": np.ascontiguousarray(W1f.T, dtype=bf16),
        "WgkvT": np.ascontiguousarray(
            np.concatenate([Wg1k.T, Wv.T], axis=1), dtype=bf16),
        "Wg1qT": np.ascontiguousarray(Wg1q.T, dtype=bf16),
        "Wd1fT": np.ascontiguousarray(Wd1f.T, dtype=bf16),
        "Wd2fT": np.ascontiguousarray(Wd2f.T, dtype=bf16),
        "Wg1fT": np.ascontiguousarray(Wg1f.T, dtype=bf16),
        "Wg2fT": np.ascontiguousarray(Wg2f.T, dtype=bf16),
        "W2fT": np.ascontiguousarray(W2f.T, dtype=bf16),
        "E": E.astype(bf16),
        "negI": (-np.eye(DM)).astype(np.float16),
        "ident": np.eye(DM, dtype=np.float32),
        "ones1": np.ones((1, PAIR), dtype=bf16),
        "bd2r": np.ascontiguousarray(bd2.reshape(1, DM), dtype=bf16),
        "bg2r": np.ascontiguousarray(bg2.reshape(1, DM), dtype=bf16),
        "iotapk": iota_f32,
        "b1": b1.reshape(DM, 1),
        "bd1": bd1.reshape(DM, 1),
        "bd2": bd2.reshape(DM, 1),
        "bg1": bg1.reshape(DM, 1),
        "bg2": bg2.reshape(DM, 1),
        "b2": b2.reshape(DP, 1),
    }

    feats = np.asarray(inputs["feats"], np.float32)
    pos = np.asarray(inputs["pos"], np.float32)
    in_maps = []
    for c in range(8):
        b, h = c // 2, c % 2
        n0 = h * NQ
        fb = feats[b]
        sq = -0.5 * (fb.astype(np.float64) ** 2).sum(axis=0)
        rhs65 = np.empty((DP + 1, N), np.float32)
        rhs65[0:DP] = fb
        rhs65[DP] = sq.astype(np.float32)
        l65 = np.empty((DP + 1, NQ), np.float32)
        l65[0:DP] = fb[:, n0:n0 + NQ]
        l65[DP] = 1.0
        biasq = (KEY_C + sq[n0:n0 + NQ]).astype(np.float32).reshape(NT, TQ).T
        m = dict(com)
        m["rhs65"] = rhs65
        m["lhsT65"] = l65
        m["feats_own"] = np.ascontiguousarray(fb[:, n0:n0 + NQ])
        m["fb_own"] = np.ascontiguousarray(fb[:, n0:n0 + NQ], dtype=bf16)
        m["feats_bf"] = np.ascontiguousarray(fb, dtype=bf16)
        m["pos_bf"] = np.ascontiguousarray(pos[b], dtype=bf16)
        m["pos_own"] = np.ascontiguousarray(pos[b][:, n0:n0 + NQ], dtype=bf16)
        m["biasq"] = np.ascontiguousarray(biasq)
        in_maps.append(m)
    return in_maps


def kernel(**inputs):
    from concourse.bass_utils import run_bass_kernel_spmd

    if "nc" not in _CACHE:
        _CACHE["nc"] = _build_bass()
    nc = _CACHE["nc"]
    in_maps = _host_prep(inputs)
    r = run_bass_kernel_spmd(nc, in_maps, core_ids=list(range(8)),
                             **_CACHE.get("run_kwargs", {}))
    _CACHE["last_result"] = r
    out = np.empty((B, DP, N), np.float32)
    for c in range(8):
        b, h = c // 2, c % 2
        out[b][:, h * NQ:(h + 1) * NQ] = r.results[c]["out"]
    return out
